# revision 19
# baseline (speedup 1.0000x reference)
"""Trainium2 Bass kernel: single-head causal attention (B=2, S=4096, E=1024, H=128).

Sharding: 8 cores = 2 batches x 4 query-quarters. Core (b, c) computes global
query tiles {c + 4m : m=0..7} (128 rows each). Each core's K/V blocks are
host-permuted so that query tile m's causal diagonal lands at local kv block
4m+3, full-attention blocks packed into slots [0, 4m+3), zero pads elsewhere.

Precision: the score path (k/q projections and q.k scores) runs 3-pass fp16
hi/lo (hh + hl + lh, ~22-bit effective) -- single-pass fp16 scores flip
contested softmax rows (sigma_score = 128, near-one-hot softmax). The value
path (vT, attention weights, attn@V) is single-pass fp16.

Schedule: x streams in 32 sub-DMAs issued first on the sync queue (DMA issue
costs ~650ns each, so order matters); weights ride the gpsimd queue, constants
the vector queue. Projections accumulate e-outer over 8 PSUM banks. Attention
tiles run big-to-small with transposes software-pipelined one tile behind
scores, and attn@V (v-stationary, wide moving operand over per-kv-block
attnT strips) interleaved as each strip group completes; outputs accumulate
in [h, q] PSUM pre-zeroed by a rank-1 null matmul, transposed back at the end.
"""
import sys
import numpy as np

if "/opt/trn_rl_repo" not in sys.path:
    sys.path.insert(0, "/opt/trn_rl_repo")

import ml_dtypes
from contextlib import ExitStack

import concourse.bass as bass
import concourse.tile as tile
from concourse import bacc, mybir
from concourse.bass_utils import run_bass_kernel_spmd

P = 128
S = 4096
E = 1024
NE = E // P          # 8 contraction tiles
NQ = 8               # q tiles per core
NKV = S // P         # 32 kv blocks
QROWS = NQ * P       # 1024 q rows per core
F32 = mybir.dt.float32
F16 = mybir.dt.float16
F8 = mybir.dt.float8e5
NEG = -1e30

_CACHE = {}


def _build():
    nc = bacc.Bacc("TRN2", target_bir_lowering=False, debug=False, num_devices=8)
    xth = nc.dram_tensor("xth", [E, S], F16, kind="ExternalInput")
    xtl = nc.dram_tensor("xtl", [E, S], F16, kind="ExternalInput")
    wqh = nc.dram_tensor("wqh", [E, P], F16, kind="ExternalInput")
    wql = nc.dram_tensor("wql", [E, P], F16, kind="ExternalInput")
    wkh = nc.dram_tensor("wkh", [E, P], F16, kind="ExternalInput")
    wkl = nc.dram_tensor("wkl", [E, P], F16, kind="ExternalInput")
    wvh = nc.dram_tensor("wvh", [E, P], F16, kind="ExternalInput")
    cbq = nc.dram_tensor("cbq", [1, P], F16, kind="ExternalInput")
    ones = nc.dram_tensor("ones", [1, 512], F16, kind="ExternalInput")
    zrow = nc.dram_tensor("zrow", [1, P], F16, kind="ExternalInput")
    bvb = nc.dram_tensor("bvb", [P, P], F32, kind="ExternalInput")
    mask0 = nc.dram_tensor("mask0", [P, 512], F32, kind="ExternalInput")
    dm512 = nc.dram_tensor("dm512", [P, 512], F32, kind="ExternalInput")
    dm1024 = nc.dram_tensor("dm1024", [P, 1024], F32, kind="ExternalInput")
    idn = nc.dram_tensor("idn", [P, P], F16, kind="ExternalInput")
    out = nc.dram_tensor("out", [QROWS, P], F32, kind="ExternalOutput")

    with tile.TileContext(nc) as tc, ExitStack() as ctx:
        # ---- persistent pools
        const = ctx.enter_context(tc.tile_pool(name="const", bufs=1))
        proj = ctx.enter_context(tc.tile_pool(name="proj", bufs=1))
        kTh = proj.tile([P, S], F16, tag="kTh")        # [h, kv]
        kTl = proj.tile([P, S], F16, tag="kTl")
        qTh = proj.tile([P, QROWS], F16, tag="qTh")    # [h, q]
        qTl = proj.tile([P, QROWS], F16, tag="qTl")
        vs = proj.tile([P, S], F16, tag="v")           # 32 blocks of [kv128, h128]

        with ExitStack() as p1:
            xp = p1.enter_context(tc.tile_pool(name="xt", bufs=1))
            wp = p1.enter_context(tc.tile_pool(name="w", bufs=1))

            # ---- DMA issue order is the schedule: x first (sync queue, 2
            # sub-DMAs per tile so arrival tracks e-order), weights on the
            # gpsimd queue, constants on the vector queue.
            xhs, xls = [], []
            for e in range(NE):
                th = xp.tile([P, S], F16, tag=f"xth{e}")
                tl = xp.tile([P, S], F16, tag=f"xtl{e}")
                for half in range(2):
                    sl = slice(half * 2048, (half + 1) * 2048)
                    nc.sync.dma_start(th[:, sl], xth.ap()[e * P:(e + 1) * P, sl])
                    nc.sync.dma_start(tl[:, sl], xtl.ap()[e * P:(e + 1) * P, sl])
                xhs.append(th)
                xls.append(tl)

            wall = {}
            for nm, dram in (("kh", wkh), ("kl", wkl), ("qh", wqh),
                             ("ql", wql), ("vh", wvh)):
                wt = wp.tile([P, NE * P], F16, tag=f"w{nm}")
                src = dram.ap()[:, :].rearrange("(e p) c -> p e c", p=P)
                dst = wt[:].rearrange("p (e c) -> p e c", e=NE)
                nc.gpsimd.dma_start(dst, src)
                wall[nm] = wt

            t_cbq = const.tile([1, P], F16, tag="cbq")
            nc.gpsimd.dma_start(t_cbq[:], cbq.ap()[:, :])
            t_ones = const.tile([1, 512], F16, tag="ones")
            nc.gpsimd.dma_start(t_ones[:], ones.ap()[:, :])
            t_zrow = const.tile([1, P], F16, tag="zrow")
            nc.gpsimd.dma_start(t_zrow[:], zrow.ap()[:, :])
            t_idn = const.tile([P, P], F16, tag="idn")
            nc.gpsimd.dma_start(t_idn[:], idn.ap()[:, :])
            t_bvb = const.tile([P, P], F32, tag="bvb")
            nc.scalar.dma_start(t_bvb[:], bvb.ap()[:, :])
            t_mask0 = const.tile([P, 512], F32, tag="mask0")
            nc.scalar.dma_start(t_mask0[:], mask0.ap()[:, :])
            t_dm512 = const.tile([P, 512], F32, tag="dm512")
            nc.scalar.dma_start(t_dm512[:], dm512.ap()[:, :])
            t_dm1024 = const.tile([P, 1024], F32, tag="dm1024")
            nc.scalar.dma_start(t_dm1024[:], dm1024.ap()[:, :])

            def w_at(nm, e):
                return wall[nm][:, bass.ts(e, P)]

            # warm the PE clock while x streams in: harmless rank-1 matmuls
            with ExitStack() as pw:
                pwu = pw.enter_context(tc.tile_pool(name="pwu", bufs=1, space="PSUM"))
                wu = pwu.tile([P, 512], F32, tag="wu")
                for i in range(10):
                    nc.tensor.matmul(wu[:], t_zrow[:], t_ones[:],
                                     start=(i == 0), stop=(i == 9))

            # kT: all 8 kv-chunks accumulate e-outer, 3-pass hi/lo (8 psum banks)
            with ExitStack() as pp:
                pk = pp.enter_context(tc.tile_pool(name="pk", bufs=8, space="PSUM"))
                pses = [pk.tile([P, 512], F32, name=f"pk{c}", tag="pk") for c in range(8)]
                for e in range(NE):
                    for c in range(8):
                        sl = bass.ts(c, 512)
                        nc.tensor.matmul(pses[c][:], w_at("kh", e), xhs[e][:, sl],
                                         start=(e == 0), stop=False)
                        nc.tensor.matmul(pses[c][:], w_at("kh", e), xls[e][:, sl],
                                         start=False, stop=False)
                    for c in range(8):
                        nc.tensor.matmul(pses[c][:], w_at("kl", e),
                                         xhs[e][:, bass.ts(c, 512)],
                                         start=False, stop=(e == NE - 1))
                for c in range(8):
                    sl = bass.ts(c, 512)
                    nc.scalar.activation(kTh[:, sl], pses[c][:],
                                         mybir.ActivationFunctionType.Copy)
                    nc.vector.tensor_sub(kTl[:, sl], pses[c][:], kTh[:, sl])

            # qT: gather diag slots {4m+3}; 2 chunks of 512, 3-pass + bias rank-1
            with ExitStack() as pp:
                pq = pp.enter_context(tc.tile_pool(name="pq", bufs=2, space="PSUM"))
                pqs = [pq.tile([P, 512], F32, name=f"pq{h}", tag="pq") for h in range(2)]
                for e in range(NE):
                    dh = xhs[e][:].rearrange("p (g f b) -> p g f b", f=4, b=P)[:, :, 3, :]
                    dl = xls[e][:].rearrange("p (g f b) -> p g f b", f=4, b=P)[:, :, 3, :]
                    for h in range(2):
                        hs = slice(h * 4, (h + 1) * 4)
                        nc.tensor.matmul(pqs[h][:], w_at("qh", e), dh[:, hs, :],
                                         start=(e == 0), stop=False)
                        nc.tensor.matmul(pqs[h][:], w_at("qh", e), dl[:, hs, :],
                                         start=False, stop=False)
                    for h in range(2):
                        nc.tensor.matmul(pqs[h][:], w_at("ql", e),
                                         dh[:, h * 4:(h + 1) * 4, :],
                                         start=False, stop=False)
                for h in range(2):
                    nc.tensor.matmul(pqs[h][:], t_cbq[:], t_ones[:], start=False, stop=True)
                    hs = bass.ts(h, 512)
                    nc.scalar.activation(qTh[:, hs], pqs[h][:],
                                         mybir.ActivationFunctionType.Copy)
                    nc.vector.tensor_sub(qTl[:, hs], pqs[h][:], qTh[:, hs])

            # vT (single-pass) then transpose into v blocks [kv, h]
            vT = proj.tile([P, S], F16, tag="vT")
            with ExitStack() as pp:
                pv = pp.enter_context(tc.tile_pool(name="pv", bufs=8, space="PSUM"))
                pses = [pv.tile([P, 512], F32, name=f"pv{c}", tag="pv") for c in range(8)]
                for e in range(NE):
                    for c in range(8):
                        nc.tensor.matmul(pses[c][:], w_at("vh", e),
                                         xhs[e][:, bass.ts(c, 512)],
                                         start=(e == 0), stop=(e == NE - 1))
                for c in range(8):
                    nc.scalar.activation(vT[:, bass.ts(c, 512)], pses[c][:],
                                         mybir.ActivationFunctionType.Copy)
            with ExitStack() as pp:
                ptv = pp.enter_context(tc.tile_pool(name="ptv", bufs=2, space="PSUM"))
                for b in range(2):  # 2 batches of 16 transposes
                    pt_ = ptv.tile([P, 16 * P], F16, tag="ptv")
                    for j in range(16):
                        nc.tensor.transpose(pt_[:, bass.ts(j, P)],
                                            vT[:, bass.ts(b * 16 + j, P)], t_idn[:])
                    nc.vector.tensor_copy(vs[:, b * 16 * P:(b + 1) * 16 * P], pt_[:])

        # ---------------- phase 2: scores + softmax + transposes + attn@V ----
        sb = ctx.enter_context(tc.tile_pool(name="sbuf2", bufs=2))
        atp = ctx.enter_context(tc.tile_pool(name="attnT", bufs=1))
        smalls = ctx.enter_context(tc.tile_pool(name="smalls", bufs=16))
        osb = ctx.enter_context(tc.tile_pool(name="osb", bufs=1))
        attnT = atp.tile([P, NKV * QROWS], F16, tag="attnT")  # strip j at [j*1024,(j+1)*1024)
        strips = attnT[:].rearrange("p (j q) -> p j q", j=NKV)

        rcps = {}
        with ExitStack() as p2:
            pob = p2.enter_context(tc.tile_pool(name="po", bufs=1, space="PSUM"))
            po = pob.tile([P, QROWS], F32, tag="po")  # [h, q] output accumulator
            p2a = p2.enter_context(ExitStack())
            pscore = p2a.enter_context(tc.tile_pool(name="ps", bufs=2, space="PSUM"))
            ptt = p2a.enter_context(tc.tile_pool(name="pt", bufs=1, space="PSUM"))

            # pre-zero po via rank-1 null matmuls; all attn@V MMs then accumulate
            for h in range(2):
                nc.tensor.matmul(po[:, bass.ts(h, 512)], t_zrow[:], t_ones[:],
                                 start=True, stop=False, skip_group_check=True)

            def emit_transposes(m, attn):
                # transpose attn blocks into strips: strip j, cols [m*128,(m+1)*128)
                nb = 4 * (m + 1)
                for b in range((nb + 15) // 16):
                    j0 = b * 16
                    jn = min(16, nb - j0)
                    pt_ = ptt.tile([P, 16 * P], F16, tag="pt")
                    for j in range(jn):
                        nc.tensor.transpose(pt_[:, bass.ts(j, P)],
                                            attn[:, bass.ts(j0 + j, P)], t_idn[:])
                    dst = strips[:, j0:j0 + jn, m * P:(m + 1) * P]
                    src = pt_[:].rearrange("p (j q) -> p j q", j=16)[:, 0:jn, :]
                    if b % 2 == 0:
                        nc.vector.tensor_copy(dst, src)
                    else:
                        nc.scalar.activation(dst, src, mybir.ActivationFunctionType.Copy)

            def emit_av_group(m, final):
                # strips j in [4m, 4m+4) are complete; accumulate into po
                for jj in range(4):
                    j = 4 * m + jj
                    q0 = P * m
                    segs = [(q0, 512), (512, 1024)] if q0 < 512 else [(q0, 1024)]
                    for si, (h0, h1) in enumerate(segs):
                        nc.tensor.matmul(po[:, h0:h1],
                                         vs[:, bass.ts(j, P)],
                                         strips[:, j, h0:h1],
                                         start=False,
                                         stop=(final and jj == 3 and si == len(segs) - 1),
                                         skip_group_check=True)

            pending = []  # software pipeline: transposes lag scores by one tile
            avq = []      # attn@V groups lag transposes by one more slot
            for m in reversed(range(NQ)):  # big tiles first; tail tile is small
                L = 512 * (m + 1)
                nchunk = (L + 1023) // 1024
                s_sb = sb.tile([P, S], F32, tag="s")
                attn = sb.tile([P, S], F16, tag="attn")
                mxp = smalls.tile([P, 4], F32, tag=f"mxp{m}")
                lqh = qTh[:, bass.ts(m, P)]
                lql = qTl[:, bass.ts(m, P)]
                for c in range(nchunk):
                    c0 = c * 1024
                    w = min(1024, L - c0)
                    ps = pscore.tile([P, 1024], F32, tag="ps")
                    for h in range(w // 512):
                        psl = ps[:, bass.ts(h, 512)]
                        ksl = slice(c0 + h * 512, c0 + (h + 1) * 512)
                        nc.tensor.matmul(psl, lqh, kTh[:, ksl], start=True, stop=False)
                        nc.tensor.matmul(psl, lqh, kTl[:, ksl], start=False, stop=False)
                    for h in range(w // 512):
                        psl = ps[:, bass.ts(h, 512)]
                        ksl = slice(c0 + h * 512, c0 + (h + 1) * 512)
                        nc.tensor.matmul(psl, lql, kTh[:, ksl], start=False, stop=True)
                    last = (c == nchunk - 1)
                    if m == 0:
                        mask = t_mask0
                    elif last:
                        mask = t_dm512 if w == 512 else t_dm1024
                    else:
                        mask = None
                    if mask is None:
                        # clean chunk: ACT copies, DVE reduces straight from PSUM
                        nc.scalar.activation(s_sb[:, c0:c0 + w], ps[:, 0:w],
                                             mybir.ActivationFunctionType.Copy)
                        nc.vector.reduce_max(mxp[:, c:c + 1], ps[:, 0:w],
                                             axis=mybir.AxisListType.X)
                    else:
                        nc.vector.tensor_add(s_sb[:, c0:c0 + w], ps[:, 0:w],
                                             mask[:, 0:w])
                        nc.vector.reduce_max(mxp[:, c:c + 1], s_sb[:, c0:c0 + w],
                                             axis=mybir.AxisListType.X)
                mx = smalls.tile([P, 1], F32, tag=f"mx{m}")
                nc.vector.reduce_max(mx[:], mxp[:, 0:nchunk], axis=mybir.AxisListType.X)
                nmx = smalls.tile([P, 1], F32, tag=f"nmx{m}")
                nc.vector.tensor_scalar_mul(nmx[:], mx[:], -1.0)
                rs = smalls.tile([P, 1], F32, tag=f"rs{m}")
                nc.scalar.activation(attn[:, :L], s_sb[:, :L],
                                     mybir.ActivationFunctionType.Exp,
                                     bias=nmx[:], scale=1.0, accum_out=rs[:])
                rcp = smalls.tile([P, 1], F32, tag=f"rcp{m}")
                nc.vector.reciprocal(rcp[:], rs[:])
                rcps[m] = rcp
                pending.append((m, attn))
                if avq:
                    emit_av_group(avq.pop(0), final=False)
                if len(pending) > 1:
                    pm, pattn = pending.pop(0)
                    emit_transposes(pm, pattn)
                    avq.append(pm)
            while pending:
                pm, pattn = pending.pop(0)
                emit_transposes(pm, pattn)
                avq.append(pm)
            while avq:
                pm = avq.pop(0)
                emit_av_group(pm, final=(not avq))
            p2a.close()  # free pscore/ptt banks before the epilogue pool opens

            # epilogue: normalize, transpose [h,q] -> [q,h], bias, store
            with ExitStack() as p3:
                pot = p3.enter_context(tc.tile_pool(name="pot", bufs=1, space="PSUM"))
                oT = osb.tile([P, QROWS], F16, tag="oT")
                nc.scalar.activation(oT[:], po[:], mybir.ActivationFunctionType.Copy)
                pout = pot.tile([P, QROWS], F16, tag="pout")
                ot = osb.tile([P, QROWS], F32, tag="ot")
                for m in range(NQ):
                    nc.tensor.transpose(pout[:, bass.ts(m, P)], oT[:, bass.ts(m, P)],
                                        t_idn[:])
                    nc.vector.scalar_tensor_tensor(ot[:, bass.ts(m, P)],
                                                   pout[:, bass.ts(m, P)],
                                                   rcps[m][:], t_bvb[:],
                                                   op0=mybir.AluOpType.mult,
                                                   op1=mybir.AluOpType.add)
                dst = out.ap()[:, :].rearrange("(m p) c -> p m c", p=P)
                src = ot[:].rearrange("p (m c) -> p m c", m=NQ)
                nc.sync.dma_start(dst, src)

    nc.compile()
    return nc


def _host_prep(input, Wq, bq, Wk, bk, Wv, bv):
    c = np.float32(np.sqrt(np.float32(P)))

    def split(a):
        a = np.asarray(a, np.float32)
        hi = a.astype(np.float16)
        lo = (a - hi.astype(np.float32)).astype(np.float16)
        return hi, lo

    wq_h, wq_l = split(np.asarray(Wq, np.float32) * c)
    wk_h, wk_l = split(Wk)
    wv_h = np.asarray(Wv, np.float32).astype(np.float16)
    cbq = (np.asarray(bq, np.float32) * c).astype(np.float16).reshape(1, P)
    # bk drops out: the (q_i+bq).bk score term is constant per query row and
    # softmax is shift-invariant per row.
    neg = np.float32(NEG)
    trilm = np.where(np.tril(np.ones((P, P), bool)), np.float32(0), neg)
    dm512 = np.zeros((P, 512), np.float32)
    dm512[:, 384:512] = trilm
    dm1024 = np.zeros((P, 1024), np.float32)
    dm1024[:, 896:1024] = trilm
    bvb = np.broadcast_to(np.asarray(bv, np.float32), (P, P)).copy()
    idn = np.eye(P, dtype=np.float16)
    ones = np.ones((1, 512), np.float16)
    zrow = np.zeros((1, P), np.float16)

    in_maps = []
    metas = []
    for core in range(8):
        b, cq = divmod(core, 4)
        assign = {}
        for m in range(NQ):
            assign[4 * m + 3] = cq + 4 * m
            if m == 0:
                for g in range(cq):
                    assign[g] = g
            else:
                for t, g in enumerate(range(cq + 4 * m - 3, cq + 4 * m)):
                    assign[4 * m + t] = g
        X = np.asarray(input[b], np.float32)
        XT = np.zeros((E, S), np.float32)
        for slot, g in assign.items():
            XT[:, slot * P:(slot + 1) * P] = X[g * P:(g + 1) * P, :].T
        xh, xl = split(XT)
        m0 = np.full((P, 512), neg, np.float32)
        m0[:, :cq * P] = 0.0
        m0[:, 384:512] = trilm
        in_maps.append({
            "xth": xh, "xtl": xl, "wqh": wq_h, "wql": wq_l,
            "wkh": wk_h, "wkl": wk_l, "wvh": wv_h, "cbq": cbq,
            "ones": ones, "zrow": zrow, "bvb": bvb, "mask0": m0,
            "dm512": dm512, "dm1024": dm1024, "idn": idn,
        })
        metas.append((b, cq))
    return in_maps, metas


def kernel(input, Wq, bq, Wk, bk, Wv, bv, _trace=False):
    if "nc" not in _CACHE:
        _CACHE["nc"] = _build()
    nc = _CACHE["nc"]
    in_maps, metas = _host_prep(np.asarray(input), np.asarray(Wq), np.asarray(bq),
                                np.asarray(Wk), np.asarray(bk),
                                np.asarray(Wv), np.asarray(bv))
    try:
        res = run_bass_kernel_spmd(nc, in_maps, list(range(8)), trace=_trace)
    except ModuleNotFoundError:
        res = run_bass_kernel_spmd(nc, in_maps, list(range(8)), trace=False)
    _CACHE["last_result"] = res
    B = 2
    full = np.zeros((B, S, P), np.float32)
    for core, (b, cq) in enumerate(metas):
        o = res.results[core]["out"]
        for m in range(NQ):
            g = cq + 4 * m
            full[b, g * P:(g + 1) * P, :] = o[m * P:(m + 1) * P, :]
    return full
